# revision 38
# baseline (speedup 1.0000x reference)
"""Trainium2 Bass kernel for BatchedAdjacency (exact Gaussian-kernel MVM).

Math per batch b (n = H*W = 4096 pixels, d = 5 guide dims, L = 16 channels):
    W[i,j]   = exp(-0.5 * ||r_i - r_j||^2)      (symmetric!)
    out[l,i] = sum_j W[i,j] * s[j,l] - s[i,l]

Distribution: 8 cores = 4 batches x 2 cores; each core computes HALF of the
symmetric W (one member of every transpose-pair of 512x512 super-blocks) and
produces a PARTIAL out[16, 4096]; the host adds the two cores' partials.

Symmetry exploitation (the big win vs computing all of W):
  - The 8x8 grid of [512,512] super-blocks has 28 off-diagonal transpose
    pairs + 8 diagonal supers.  A core owns 14 pairs + 4 diagonals.  For an
    owned pair {a<b} it computes T1 = W[j in b, i in a] once via MM1+exp,
    feeds acc[a] directly, and obtains the mirrored position (a,b)
    via PE-transpose (128x128 fp16 identity matmuls) + DVE
    PSUM->SBUF copies, feeding acc[b] later.  exp volume drops ~2x (ACT was
    co-critical with PE in the non-symmetric version).
  - SPMD trick: both cores of a batch run the SAME program; core parity h
    relabels super-columns by v -> (v+h) % 8 in its host-prepped inputs.
    The owned virtual pair set {u, u+d mod 8} (d=1,2,3, u even) + {0,4},{2,6}
    and virtual diagonals {0,2,4,6} tile the physical pair/diag sets exactly
    across the two parities.  The parity-0 core alone subtracts src (the
    identity term); the host ADDS the two partials.

Device pipeline per core (matmuls fp16 with hi/lo splits, PSUM fp32):
  - MM1 computes -0.5*d2 tiles [j=128, i=512] via augmented features (K=19);
    two j-tiles packed into 32-row PE strips fill a 2-bank PSUM duo.
  - ScalarE exp() on duos PSUM->SBUF fp16.
  - PE transposes mirror each pair's 4 tiles (16 blocks) into 2 PSUM banks;
    DVE copies them to the persistent T2 SBUF buffer.
  - MM2 accumulates acc[c][48, 512] over the 8 chunk windows sequentially
    (acc double-buffered in 2 PSUM banks; duos 4; transposes 2).
  - VectorE drain: out = acc_hi + acc_lo - src, DMA to HBM.
"""

import sys

if "/opt/trn_rl_repo" not in sys.path:
    sys.path.insert(0, "/opt/trn_rl_repo")

import numpy as np

import concourse.bacc as bacc
import concourse.bass as bass
import concourse.mybir as mybir
import concourse.tile as tile
from concourse.bass_utils import run_bass_kernel_spmd
from concourse.masks import make_identity


def install_ntff_hook() -> None:
    """Provide antenv.axon_hooks (absent in this image) so that
    run_bass_kernel_spmd can profile via the axon .so when tracing is
    requested -- and so a stray BASS_TRACE env var cannot crash the run."""
    import types

    if "antenv.axon_hooks" in sys.modules:
        return
    hook = None
    try:
        import antenv
        from trn_agent_boot.trn_boot import _ntff_profile_via_ctypes

        hook = _ntff_profile_via_ctypes("/opt/axon/libaxon_pjrt.so")
    except Exception:
        antenv = None
    mod = types.ModuleType("antenv.axon_hooks")
    mod._hook = hook
    mod.get_axon_ntff_profile_hook = lambda: mod._hook
    mod.set_axon_ntff_profile_hook = lambda h: setattr(mod, "_hook", h)
    sys.modules["antenv.axon_hooks"] = mod
    if antenv is not None:
        antenv.axon_hooks = mod


install_ntff_hook()

BS, L, D, H, W = 4, 16, 5, 64, 64
N = H * W            # 4096 pixels
NCORES = 8
CHUNK = 512          # i-tile (PSUM bank / fp32 matmul free-dim limit)
NCHUNK = N // CHUNK  # 8 super-columns, all owned by every core (partial sums)
JB = 128             # j-block (contraction tile)
NJB = N // JB        # 32
KAUG = 19            # augmented feature count (fp16 path, unused)
K8 = 18              # fp8 DoubleRow lanes per half (36 features total)
GS = 2               # j-tiles packed per PSUM duo tile

FP16 = mybir.dt.float16
FP8 = mybir.dt.float8e4
F32 = mybir.dt.float32

# ---- virtual super-block ownership (same for every core; the host permutes
# ---- columns per core parity so the union covers the full symmetric grid).
VPAIRS = []
for d in (1, 2, 3):
    for u in (0, 2, 4, 6):
        a, b = u, (u + d) % 8
        VPAIRS.append((min(a, b), max(a, b)))
VPAIRS += [(0, 4), (2, 6)]
VDIAGS = [0, 2, 4, 6]
# pairs whose mirror is RECOMPUTED by a second MM1+exp at window b instead of
# PE-transposed: per pair this trades 1.66us of PE transposes for 0.85us of
# MM1 plus ~2.1us of exp on the slack ScalarE (ACT stays ~20us under PE)
RECOMP = {(0, 1), (2, 3)}
# pairs whose W tiles are exp'd straight to fp8e4m3 and consumed by
# DoubleRow MM2 (2 j-blocks per instruction, ~2x MM2 rate on those tiles).
# 4 pairs = 16/64 supers across the two parities; exact-input simulation
# puts the end-to-end rel err at 1.33e-2 vs the 2e-2 gate.
FP8_PAIRS = {(2, 4), (2, 6), (4, 6), (2, 5)}
assert FP8_PAIRS <= set(VPAIRS) and not (FP8_PAIRS & RECOMP)

# production groups per window (pairs {a,b} are produced at window a as
# T1 = W[j in b's blocks, i in a's chunk]); diag c produced at window c.
PROD_GROUPS = []
for c in range(NCHUNK):
    if c in VDIAGS:
        PROD_GROUPS.append(("diag", c, c))
    for (a, b) in VPAIRS:
        if a == c:
            PROD_GROUPS.append(("pair", a, b))
    for (a, b) in sorted(RECOMP):
        if b == c:
            # recomputed mirror position (a, b): tiles (4a+t, i-chunk b)
            PROD_GROUPS.append(("rmirr", b, a))

# flat production tile stream: tile = (vjb, ichunk, group_index); groups are
# 4 tiles so duo (GS=2) boundaries never straddle a group asymmetrically
PROD_TILES = []
for gi, (kind, a, b) in enumerate(PROD_GROUPS):
    for t in range(4):
        PROD_TILES.append((4 * b + t, a, gi))
NT = len(PROD_TILES)            # 72
NDUO = NT // GS                 # 36
assert NT == GS * NDUO


WARMUP_FILLERS = 5
BRIDGE_FILLERS = 5


def build_nc() -> bass.Bass:
    nc = bacc.Bacc()

    # aug tensors carry the 19 features replicated in rows 0:19 and 32:51 of
    # a 64-partition layout (wide DMAs: narrow-partition transfers only get
    # partition_count/128 of DMA bandwidth).
    aug_j = nc.declare_dram_parameter("aug_j", [64, N], FP16, isOutput=False)
    aug_i = nc.declare_dram_parameter("aug_i", [64, N], FP16, isOutput=False)
    s_rows = nc.declare_dram_parameter("s_rows", [128, NJB * 48], FP16, isOutput=False)
    s_rows8 = nc.declare_dram_parameter(
        "s_rows8", [128, NJB * 48], FP8, isOutput=False
    )
    s_nat = nc.declare_dram_parameter("s_nat", [16, N], FP16, isOutput=False)
    out = nc.declare_dram_parameter("out", [16, N], F32, isOutput=True)

    def group_dt(gi: int):
        kind, a, b = PROD_GROUPS[gi]
        if kind == "pair" and (a, b) in FP8_PAIRS:
            return FP8
        if kind == "diag":
            # diag supers consumed direct-only; fp8+DR there too.  Exact-input
            # sim: pairs+diags end-to-end rel err 1.63e-2 (gate 2e-2).
            return FP8
        return FP16

    with tile.TileContext(nc) as tc:
        with (
            tc.tile_pool(name="const", bufs=1) as cpool,
            tc.tile_pool(name="t2pool", bufs=1) as t2pool,
            tc.tile_pool(name="wpool", bufs=10) as wpool,
            tc.tile_pool(name="ppool", bufs=2, space="PSUM") as ppool,
            tc.tile_pool(name="apool", bufs=2, space="PSUM") as apool,
            tc.tile_pool(name="tpool", bufs=2, space="PSUM") as tpool,
            tc.tile_pool(name="opool", bufs=2) as opool,
        ):
            aug_j_sb = cpool.tile([64, N], FP16)
            aug_i_sb = cpool.tile([64, N], FP16)
            s_rows_sb = cpool.tile([128, NJB * 48], FP16)
            s_rows8_sb = cpool.tile([128, NJB, 48], FP8)
            s_nat_sb = cpool.tile([16, N], FP16)
            # input DMAs spread over the 3 DMA-capable queues (sync, scalar,
            # gpsimd), ordered so window 0's operands land first: aug_i
            # chunk 0, aug_j in consumption order, s_rows; the rest trails.
            # first pieces sized >=1KB per partition line (smaller lines lose
            # DMA efficiency); dependency granularity = one dma_start
            # dma_start triggers occupy the ISSUING engine's queue (~0.6us
            # each): scalar (= ACT, the production pacer) gets only 2 early
            # triggers; sync and gpsimd carry the rest, ordered by need-time.
            nc.scalar.dma_start(out=aug_j_sb[:, :512], in_=aug_j[:, :512])
            nc.scalar.dma_start(out=s_rows8_sb[:, :8, :], in_=s_rows8[:, : 8 * 48])
            nc.sync.dma_start(out=aug_i_sb[:, :CHUNK], in_=aug_i[:, :CHUNK])
            nc.gpsimd.dma_start(out=s_rows_sb[:, :384], in_=s_rows[:, :384])
            nc.sync.dma_start(out=aug_j_sb[:, 512:1024], in_=aug_j[:, 512:1024])
            nc.sync.dma_start(out=aug_j_sb[:, 1024:1536], in_=aug_j[:, 1024:1536])
            nc.gpsimd.dma_start(out=s_rows_sb[:, 1152:], in_=s_rows[:, 1152:])
            nc.sync.dma_start(out=aug_j_sb[:, 1536:2048], in_=aug_j[:, 1536:2048])
            nc.gpsimd.dma_start(out=s_rows8_sb[:, 8:, :], in_=s_rows8[:, 8 * 48 :])
            nc.sync.dma_start(out=s_rows_sb[:, 384:768], in_=s_rows[:, 384:768])
            nc.sync.dma_start(out=aug_j_sb[:, 2048:2560], in_=aug_j[:, 2048:2560])
            nc.gpsimd.dma_start(out=aug_j_sb[:, 2560:3072], in_=aug_j[:, 2560:3072])
            nc.sync.dma_start(out=s_rows_sb[:, 768:1152], in_=s_rows[:, 768:1152])
            nc.gpsimd.dma_start(out=aug_j_sb[:, 3584:], in_=aug_j[:, 3584:])
            nc.sync.dma_start(out=aug_j_sb[:, 3072:3584], in_=aug_j[:, 3072:3584])
            nc.gpsimd.dma_start(out=aug_i_sb[:, 1024:2048], in_=aug_i[:, 1024:2048])
            nc.sync.dma_start(out=aug_i_sb[:, CHUNK:1024], in_=aug_i[:, CHUNK:1024])
            nc.sync.dma_start(out=s_nat_sb[:], in_=s_nat[:])
            nc.gpsimd.dma_start(out=aug_i_sb[:, 2048:], in_=aug_i[:, 2048:])

            ident = cpool.tile([128, 128], FP16)
            make_identity(nc, ident[:])
            ident8 = cpool.tile([128, 128], FP8)
            make_identity(nc, ident8[:])

            # persistent transposed-mirror buffers, one per owned pair
            t2_tiles = {}
            for (a, b) in VPAIRS:
                dt = FP8 if (a, b) in FP8_PAIRS else FP16
                t2_tiles[(a, b)] = t2pool.tile(
                    [128, 4, CHUNK], dt, name=f"t2_{a}{b}"
                )

            zt = cpool.tile([128, 640], FP16)
            nc.vector.memset(zt[:], 0.0)
            # dedicated spare PSUM slot for warm-up/filler matmuls (shares the
            # acc tag; real accumulators rotate through the other slot first)
            wz = apool.tile([128, CHUNK], F32, tag="acc", name="wz")

            def filler(n):
                for _ in range(n):
                    nc.tensor.matmul(
                        wz[:],
                        lhsT=zt[:, :128],
                        rhs=zt[:, 128:640],
                        start=True,
                        stop=True,
                    )

            # PE warm-up during input DMAs so HAM reaches full clock before
            # the first real MM1; sized to end right as the bulk input lands
            filler(WARMUP_FILLERS)

            # ---------------- production machinery ----------------
            wt_tiles = [None] * NDUO        # duo idx -> SBUF fp16 tile
            duo_emitted = 0
            group_tiles_expd = [0] * len(PROD_GROUPS)

            def wt_slice(ti, lo, hi):
                """W-values slice [128, lo:hi] of production tile ti."""
                d, slot = divmod(ti, GS)
                return wt_tiles[d][:, slot, lo:hi]

            def emit_transposes(gi):
                kind, a, b = PROD_GROUPS[gi]
                if kind != "pair" or (a, b) in RECOMP:
                    return
                t2 = t2_tiles[(a, b)]
                fp8 = (a, b) in FP8_PAIRS
                # T2 tile p covers [j in block 4a+p, i in chunk b]; its 128-col
                # sub-block q is the transpose of production tile q's columns
                # p*128:(p+1)*128.  Two p-slices batch into one PSUM bank.
                # fp8 transposes must write PSUM with element step 2.
                for half in range(2):
                    if fp8:
                        tp = tpool.tile([128, 2, 2 * CHUNK], FP8, tag="tp", name="tp")
                    else:
                        tp = tpool.tile([128, 2, CHUNK], FP16, tag="tp", name="tp")
                    for pp in range(2):
                        p = 2 * half + pp
                        for q in range(4):
                            if fp8:
                                dst = tp[:, pp, 2 * q * JB : 2 * (q + 1) * JB : 2]
                            else:
                                dst = tp[:, pp, q * JB : (q + 1) * JB]
                            nc.tensor.transpose(
                                dst,
                                wt_slice(4 * gi + q, p * JB, (p + 1) * JB),
                                ident8[:] if fp8 else ident[:],
                            )
                    src = tp[:, :, ::2] if fp8 else tp[:]
                    nc.vector.tensor_copy(
                        out=t2[:, 2 * half : 2 * half + 2, :], in_=src
                    )

            def emit_duo():
                nonlocal duo_emitted
                t = duo_emitted
                duo_emitted += 1
                gi_t = PROD_TILES[GS * t][2]
                p = ppool.tile([128, GS, CHUNK], F32, tag="p", name="p")
                for s in range(GS):
                    vjb, ic, _gi = PROD_TILES[GS * t + s]
                    nc.tensor.matmul(
                        p[:, s, :],
                        lhsT=aug_j_sb[32 * s : 32 * s + KAUG, vjb * JB : (vjb + 1) * JB],
                        rhs=aug_i_sb[32 * s : 32 * s + KAUG, ic * CHUNK : (ic + 1) * CHUNK],
                        start=True,
                        stop=True,
                    )
                wt = wpool.tile([128, GS, CHUNK], group_dt(gi_t), tag="w", name="wt")
                nc.scalar.activation(
                    wt[:], p[:], mybir.ActivationFunctionType.Exp
                )
                wt_tiles[t] = wt
                # emit PE transposes for any pair group fully exp'd now
                for s in range(GS):
                    _vjb, _ic, gi = PROD_TILES[GS * t + s]
                    group_tiles_expd[gi] += 1
                    if group_tiles_expd[gi] == 4:
                        emit_transposes(gi)

            def ensure_produced(tile_idx, lookahead=1):
                want = min(tile_idx // GS + 1 + lookahead, NDUO)
                while duo_emitted < want:
                    emit_duo()

            # ---------------- consumption schedule ----------------
            mirrors_at = {c: [] for c in range(NCHUNK)}
            for (a, b) in VPAIRS:
                if (a, b) not in RECOMP:
                    mirrors_at[b].append((a, b))

            mm2_since_duo = [0]

            def maybe_pull_ahead():
                # keep ACT fed during windows with little/no production
                if duo_emitted < NDUO and mm2_since_duo[0] >= 2:
                    emit_duo()
                    mm2_since_duo[0] = 0

            for c in range(NCHUNK):
                # entries: ("direct", vjb, ti) fp16 per-tile;
                #          ("mirror", vjb, (a,b,p)) fp16 per-tile;
                #          ("dr", duo_t, pr) fp8 DoubleRow direct per-duo;
                #          ("drm", (a,b,u), pr) fp8 DoubleRow mirror per-duo.
                mm2s = []
                for gi, (kind, a, b) in enumerate(PROD_GROUPS):
                    if a != c:
                        continue
                    if group_dt(gi) == FP8:
                        for u in range(2):
                            duo_t = 2 * gi + u
                            vjb0 = PROD_TILES[GS * duo_t][0]
                            mm2s.append(("dr", duo_t, vjb0))
                    else:
                        for q in range(4):
                            ti = 4 * gi + q
                            vjb, _ic, _g = PROD_TILES[ti]
                            mm2s.append(("direct", vjb, ti))
                for (a, b) in mirrors_at[c]:
                    if (a, b) in FP8_PAIRS:
                        for u in range(2):
                            mm2s.append(("drm", (a, b, u), 4 * a + 2 * u))
                    else:
                        for p in range(4):
                            mm2s.append(("mirror", 4 * a + p, (a, b, p)))

                acc = apool.tile([48, CHUNK], F32, tag="acc", name="acc")
                nmm = len(mm2s)
                for k, (kind, ref, aux) in enumerate(mm2s):
                    if c == 0 and k == 0:
                        filler(BRIDGE_FILLERS)  # bridge the pipeline-fill PE gap
                    if kind == "direct":
                        vjb, ti = ref, aux
                        ensure_produced(ti)
                        nc.tensor.matmul(
                            acc[:],
                            lhsT=s_rows_sb[:, vjb * 48 : (vjb + 1) * 48],
                            rhs=wt_slice(ti, 0, CHUNK),
                            start=(k == 0),
                            stop=(k == nmm - 1),
                            skip_group_check=True,
                        )
                    elif kind == "mirror":
                        vjb = ref
                        a, b, p = aux
                        nc.tensor.matmul(
                            acc[:],
                            lhsT=s_rows_sb[:, vjb * 48 : (vjb + 1) * 48],
                            rhs=t2_tiles[(a, b)][:, p, :],
                            start=(k == 0),
                            stop=(k == nmm - 1),
                            skip_group_check=True,
                        )
                    elif kind == "dr":
                        duo_t, vjb0 = ref, aux
                        ensure_produced(GS * duo_t + 1)
                        nc.tensor.matmul(
                            acc[:],
                            lhsT=s_rows8_sb[:, vjb0 : vjb0 + 2, :],
                            rhs=wt_tiles[duo_t][:],
                            start=(k == 0),
                            stop=(k == nmm - 1),
                            perf_mode=mybir.MatmulPerfMode.DoubleRow,
                            skip_group_check=True,
                        )
                    else:  # drm
                        a, b, u = ref
                        vjb0 = aux
                        nc.tensor.matmul(
                            acc[:],
                            lhsT=s_rows8_sb[:, vjb0 : vjb0 + 2, :],
                            rhs=t2_tiles[(a, b)][:, 2 * u : 2 * u + 2, :],
                            start=(k == 0),
                            stop=(k == nmm - 1),
                            perf_mode=mybir.MatmulPerfMode.DoubleRow,
                            skip_group_check=True,
                        )
                    mm2_since_duo[0] += 2 if kind in ("dr", "drm") else 1
                    maybe_pull_ahead()

                isl = slice(c * CHUNK, (c + 1) * CHUNK)
                has_fp8 = any(kind in ("dr", "drm") for kind, _r, _a in mm2s)
                if has_fp8:
                    # acc rows 32:48 carry the fp8 s-lo partials
                    t0 = opool.tile([16, CHUNK], F32, tag="t0", name="t0")
                    nc.vector.tensor_sub(t0[:], acc[0:16, :], s_nat_sb[:, isl])
                    o = opool.tile([16, CHUNK], F32, tag="o", name="o")
                    nc.vector.tensor_add(o[:], t0[:], acc[32:48, :])
                else:
                    # fp16 s-lo columns are zeroed: single-op drain
                    o = opool.tile([16, CHUNK], F32, tag="o", name="o")
                    nc.vector.tensor_sub(o[:], acc[0:16, :], s_nat_sb[:, isl])
                nc.sync.dma_start(out=out[:, isl], in_=o[:])

    nc.finalize()
    return nc


def _hi_lo(x: np.ndarray):
    hi = x.astype(np.float16)
    lo = (x - hi.astype(np.float32)).astype(np.float16)
    return hi, lo


def _tri8(x: np.ndarray):
    """Triple-split into e4m3 levels: x ~= a + b + c with rel err ~2^-12."""
    import ml_dtypes

    e4 = ml_dtypes.float8_e4m3
    a = x.astype(e4)
    b = (x - a.astype(np.float32)).astype(e4)
    c = (x - a.astype(np.float32) - b.astype(np.float32)).astype(e4)
    return a, b, c


def _pack_dr_features(lhs36: np.ndarray, rhs36: np.ndarray):
    """Pack [36, N] fp8 feature matrices into the DoubleRow SBUF layouts:
    aug_j8[32s+p, vjb*256 + h*128 + c] = lhs36[18h+p, vjb*128+c]  (strips s=0,1)
    aug_i8[32s+p, ic*1024 + h*512 + c] = rhs36[18h+p, ic*512+c]."""
    import ml_dtypes

    e4 = ml_dtypes.float8_e4m3
    n = lhs36.shape[1]
    aj = np.zeros((128, 2 * n), e4)
    ai = np.zeros((128, 2 * n), e4)
    A = lhs36.reshape(2, 18, n // 128, 128)        # [h, p, blk, c]
    Aj = np.transpose(A, (1, 2, 0, 3)).reshape(18, 2 * n)
    B = rhs36.reshape(2, 18, n // 512, 512)        # [h, p, chunk, c]
    Bi = np.transpose(B, (1, 2, 0, 3)).reshape(18, 2 * n)
    for s in range(2):
        aj[32 * s : 32 * s + 18] = Aj
        ai[32 * s : 32 * s + 18] = Bi
    return aj, ai


def prep_core_inputs(src: np.ndarray, guide: np.ndarray) -> list[dict]:
    """Shard full inputs into the 8 per-core input maps (host-side layout
    prep).  Core parity h sees super-columns permuted by v -> (v+h) % 8."""
    in_maps = []
    for bi in range(BS):
        refs = np.ascontiguousarray(guide[bi].reshape(D, N), dtype=np.float32)
        srcs = np.ascontiguousarray(src[bi].reshape(L, N), dtype=np.float32)
        sq = (refs.astype(np.float64) ** 2).sum(0)
        r_hi, r_lo = _hi_lo(refs)
        q_hi, q_lo = _hi_lo((-0.5 * sq).astype(np.float32))
        ones = np.ones((1, N), np.float16)
        augj = np.concatenate(
            [r_hi, r_lo, r_hi, q_hi[None], q_lo[None], ones, ones], axis=0
        )
        augi = np.concatenate(
            [r_hi, r_hi, r_lo, ones, ones, q_hi[None], q_lo[None]], axis=0
        )
        import ml_dtypes

        e4 = ml_dtypes.float8_e4m3fn
        s_hi, s_lo = _hi_lo(srcs)
        s8h = srcs.astype(e4)
        s8l = (srcs - s8h.astype(np.float32)).astype(e4)
        s_rows_phys = np.zeros((128, NJB * 48), np.float16)
        s_rows8_phys = np.zeros((128, NJB * 48), e4)
        for jb in range(NJB):
            blk = slice(jb * JB, (jb + 1) * JB)
            s_rows_phys[:, 48 * jb : 48 * jb + 16] = s_hi[:, blk].T
            s_rows8_phys[:, 48 * jb : 48 * jb + 16] = s8h[:, blk].T
            s_rows8_phys[:, 48 * jb + 32 : 48 * jb + 48] = s8l[:, blk].T
        for h in range(2):
            perm = [(v + h) % NCHUNK for v in range(NCHUNK)]  # virtual->physical
            pix = np.concatenate(
                [np.arange(p * CHUNK, (p + 1) * CHUNK) for p in perm]
            )
            augj_v = np.zeros((64, N), np.float16)
            augi_v = np.zeros((64, N), np.float16)
            for st in range(2):
                augj_v[32 * st : 32 * st + KAUG] = augj[:, pix]
                augi_v[32 * st : 32 * st + KAUG] = augi[:, pix]
            s_rows_v = np.zeros((128, NJB * 48), np.float16)
            s_rows8_v = np.zeros((128, NJB * 48), e4)
            for v in range(NCHUNK):
                pcol = perm[v]
                s_rows_v[:, v * 4 * 48 : (v + 1) * 4 * 48] = s_rows_phys[
                    :, pcol * 4 * 48 : (pcol + 1) * 4 * 48
                ]
                s_rows8_v[:, v * 4 * 48 : (v + 1) * 4 * 48] = s_rows8_phys[
                    :, pcol * 4 * 48 : (pcol + 1) * 4 * 48
                ]
            # the identity term -src is subtracted by the parity-0 core only
            # (host ADDS the two cores' partials)
            s_nat_v = (
                np.ascontiguousarray(srcs[:, pix]).astype(np.float16)
                if h == 0
                else np.zeros((L, N), np.float16)
            )
            in_maps.append(
                {
                    "aug_j": augj_v,
                    "aug_i": augi_v,
                    "s_rows": s_rows_v,
                    "s_rows8": s_rows8_v,
                    "s_nat": s_nat_v,
                }
            )
    return in_maps


_NC_CACHE = None


def _get_nc() -> bass.Bass:
    global _NC_CACHE
    if _NC_CACHE is None:
        _NC_CACHE = build_nc()
    return _NC_CACHE


def run_on_hw(in_maps, **kwargs):
    return run_bass_kernel_spmd(_get_nc(), in_maps, core_ids=list(range(NCORES)), **kwargs)


def assemble_output(results: list[dict]) -> np.ndarray:
    out = np.zeros((BS, L, N), np.float32)
    for bi in range(BS):
        for h in range(2):
            perm = [(v + h) % NCHUNK for v in range(NCHUNK)]
            part = results[2 * bi + h]["out"]
            for v in range(NCHUNK):
                p = perm[v]
                out[bi, :, p * CHUNK : (p + 1) * CHUNK] += part[
                    :, v * CHUNK : (v + 1) * CHUNK
                ]
    return out.reshape(BS, L, H, W)


def kernel(src_imgs: np.ndarray, guide_imgs: np.ndarray) -> np.ndarray:
    src = np.asarray(src_imgs, dtype=np.float32)
    guide = np.asarray(guide_imgs, dtype=np.float32)
    in_maps = prep_core_inputs(src, guide)
    res = run_on_hw(in_maps)
    return assemble_output(res.results)



# revision 39
# speedup vs baseline: 1.0345x; 1.0345x over previous
"""Trainium2 Bass kernel for BatchedAdjacency (exact Gaussian-kernel MVM).

Math per batch b (n = H*W = 4096 pixels, d = 5 guide dims, L = 16 channels):
    W[i,j]   = exp(-0.5 * ||r_i - r_j||^2)      (symmetric!)
    out[l,i] = sum_j W[i,j] * s[j,l] - s[i,l]

Distribution: 8 cores = 4 batches x 2 cores; each core computes HALF of the
symmetric W (one member of every transpose-pair of 512x512 super-blocks) and
produces a PARTIAL out[16, 4096]; the host adds the two cores' partials.

Symmetry exploitation (the big win vs computing all of W):
  - The 8x8 grid of [512,512] super-blocks has 28 off-diagonal transpose
    pairs + 8 diagonal supers.  A core owns 14 pairs + 4 diagonals.  For an
    owned pair {a<b} it computes T1 = W[j in b, i in a] once via MM1+exp,
    feeds acc[a] directly, and obtains the mirrored position (a,b)
    via PE-transpose (128x128 fp16 identity matmuls) + DVE
    PSUM->SBUF copies, feeding acc[b] later.  exp volume drops ~2x (ACT was
    co-critical with PE in the non-symmetric version).
  - SPMD trick: both cores of a batch run the SAME program; core parity h
    relabels super-columns by v -> (v+h) % 8 in its host-prepped inputs.
    The owned virtual pair set {u, u+d mod 8} (d=1,2,3, u even) + {0,4},{2,6}
    and virtual diagonals {0,2,4,6} tile the physical pair/diag sets exactly
    across the two parities.  The parity-0 core alone subtracts src (the
    identity term); the host ADDS the two partials.

Device pipeline per core (matmuls fp16 with hi/lo splits, PSUM fp32):
  - MM1 computes -0.5*d2 tiles [j=128, i=512] via augmented features (K=19);
    two j-tiles packed into 32-row PE strips fill a 2-bank PSUM duo.
  - ScalarE exp() on duos PSUM->SBUF fp16.
  - PE transposes mirror each pair's 4 tiles (16 blocks) into 2 PSUM banks;
    DVE copies them to the persistent T2 SBUF buffer.
  - MM2 accumulates acc[c][48, 512] over the 8 chunk windows sequentially
    (acc double-buffered in 2 PSUM banks; duos 4; transposes 2).
  - VectorE drain: out = acc_hi + acc_lo - src, DMA to HBM.
"""

import sys

if "/opt/trn_rl_repo" not in sys.path:
    sys.path.insert(0, "/opt/trn_rl_repo")

import numpy as np

import concourse.bacc as bacc
import concourse.bass as bass
import concourse.mybir as mybir
import concourse.tile as tile
from concourse.bass_utils import run_bass_kernel_spmd
from concourse.masks import make_identity


def install_ntff_hook() -> None:
    """Provide antenv.axon_hooks (absent in this image) so that
    run_bass_kernel_spmd can profile via the axon .so when tracing is
    requested -- and so a stray BASS_TRACE env var cannot crash the run."""
    import types

    if "antenv.axon_hooks" in sys.modules:
        return
    hook = None
    try:
        import antenv
        from trn_agent_boot.trn_boot import _ntff_profile_via_ctypes

        hook = _ntff_profile_via_ctypes("/opt/axon/libaxon_pjrt.so")
    except Exception:
        antenv = None
    mod = types.ModuleType("antenv.axon_hooks")
    mod._hook = hook
    mod.get_axon_ntff_profile_hook = lambda: mod._hook
    mod.set_axon_ntff_profile_hook = lambda h: setattr(mod, "_hook", h)
    sys.modules["antenv.axon_hooks"] = mod
    if antenv is not None:
        antenv.axon_hooks = mod


install_ntff_hook()

BS, L, D, H, W = 4, 16, 5, 64, 64
N = H * W            # 4096 pixels
NCORES = 8
CHUNK = 512          # i-tile (PSUM bank / fp32 matmul free-dim limit)
NCHUNK = N // CHUNK  # 8 super-columns, all owned by every core (partial sums)
JB = 128             # j-block (contraction tile)
NJB = N // JB        # 32
KAUG = 19            # augmented feature count (fp16 path, unused)
K8 = 18              # fp8 DoubleRow lanes per half (36 features total)
GS = 2               # j-tiles packed per PSUM duo tile

FP16 = mybir.dt.float16
FP8 = mybir.dt.float8e4
F32 = mybir.dt.float32

# ---- virtual super-block ownership (same for every core; the host permutes
# ---- columns per core parity so the union covers the full symmetric grid).
VPAIRS = []
for d in (1, 2, 3):
    for u in (0, 2, 4, 6):
        a, b = u, (u + d) % 8
        VPAIRS.append((min(a, b), max(a, b)))
VPAIRS += [(0, 4), (2, 6)]
VDIAGS = [0, 2, 4, 6]
# pairs whose mirror is RECOMPUTED by a second MM1+exp at window b instead of
# PE-transposed: per pair this trades 1.66us of PE transposes for 0.85us of
# MM1 plus ~2.1us of exp on the slack ScalarE (ACT stays ~20us under PE)
RECOMP = {(0, 1), (2, 3)}
# pairs whose W tiles are exp'd straight to fp8e4m3 and consumed by
# DoubleRow MM2 (2 j-blocks per instruction, ~2x MM2 rate on those tiles).
# 4 pairs = 16/64 supers across the two parities; exact-input simulation
# puts the end-to-end rel err at 1.33e-2 vs the 2e-2 gate.
FP8_PAIRS = {(2, 4), (2, 6), (4, 6), (2, 5)}
assert FP8_PAIRS <= set(VPAIRS) and not (FP8_PAIRS & RECOMP)

# production groups per window (pairs {a,b} are produced at window a as
# T1 = W[j in b's blocks, i in a's chunk]); diag c produced at window c.
PROD_GROUPS = []
for c in range(NCHUNK):
    if c in VDIAGS:
        PROD_GROUPS.append(("diag", c, c))
    for (a, b) in VPAIRS:
        if a == c:
            PROD_GROUPS.append(("pair", a, b))
    for (a, b) in sorted(RECOMP):
        if b == c:
            # recomputed mirror position (a, b): tiles (4a+t, i-chunk b)
            PROD_GROUPS.append(("rmirr", b, a))

# flat production tile stream: tile = (vjb, ichunk, group_index); groups are
# 4 tiles so duo (GS=2) boundaries never straddle a group asymmetrically
PROD_TILES = []
for gi, (kind, a, b) in enumerate(PROD_GROUPS):
    for t in range(4):
        PROD_TILES.append((4 * b + t, a, gi))
NT = len(PROD_TILES)            # 72
NDUO = NT // GS                 # 36
assert NT == GS * NDUO


WARMUP_FILLERS = 5
BRIDGE_FILLERS = 5


def build_nc() -> bass.Bass:
    nc = bacc.Bacc()

    # aug tensors carry the 19 features replicated in rows 0:19 and 32:51 of
    # a 64-partition layout (wide DMAs: narrow-partition transfers only get
    # partition_count/128 of DMA bandwidth).
    aug_j = nc.declare_dram_parameter("aug_j", [64, N], FP16, isOutput=False)
    aug_i = nc.declare_dram_parameter("aug_i", [64, N], FP16, isOutput=False)
    s_rows = nc.declare_dram_parameter("s_rows", [128, NJB * 48], FP16, isOutput=False)
    s_rows8 = nc.declare_dram_parameter(
        "s_rows8", [128, NJB * 48], FP8, isOutput=False
    )
    s_nat = nc.declare_dram_parameter("s_nat", [16, N], FP16, isOutput=False)
    out = nc.declare_dram_parameter("out", [16, N], F32, isOutput=True)

    def group_dt(gi: int):
        kind, a, b = PROD_GROUPS[gi]
        if kind == "pair" and (a, b) in FP8_PAIRS:
            return FP8
        if kind == "diag":
            # diag supers consumed direct-only; fp8+DR there too.  Exact-input
            # sim: pairs+diags end-to-end rel err 1.63e-2 (gate 2e-2).
            return FP8
        return FP16

    with tile.TileContext(nc) as tc:
        with (
            tc.tile_pool(name="const", bufs=1) as cpool,
            tc.tile_pool(name="t2pool", bufs=1) as t2pool,
            tc.tile_pool(name="wpool", bufs=10) as wpool,
            tc.tile_pool(name="ppool", bufs=2, space="PSUM") as ppool,
            tc.tile_pool(name="apool", bufs=2, space="PSUM") as apool,
            tc.tile_pool(name="tpool", bufs=2, space="PSUM") as tpool,
            tc.tile_pool(name="opool", bufs=2) as opool,
        ):
            aug_j_sb = cpool.tile([64, N], FP16)
            aug_i_sb = cpool.tile([64, N], FP16)
            s_rows_sb = cpool.tile([128, NJB * 48], FP16)
            s_rows8_sb = cpool.tile([128, NJB, 48], FP8)
            s_nat_sb = cpool.tile([16, N], FP16)
            # input DMAs spread over the 3 DMA-capable queues (sync, scalar,
            # gpsimd), ordered so window 0's operands land first: aug_i
            # chunk 0, aug_j in consumption order, s_rows; the rest trails.
            # first pieces sized >=1KB per partition line (smaller lines lose
            # DMA efficiency); dependency granularity = one dma_start
            # dma_start triggers occupy the ISSUING engine's queue (~0.6us
            # each): scalar (= ACT, the production pacer) gets only 2 early
            # triggers; sync and gpsimd carry the rest, ordered by need-time.
            nc.scalar.dma_start(out=aug_j_sb[:, :512], in_=aug_j[:, :512])
            nc.scalar.dma_start(out=s_rows8_sb[:, :8, :], in_=s_rows8[:, : 8 * 48])
            nc.sync.dma_start(out=aug_i_sb[:, :CHUNK], in_=aug_i[:, :CHUNK])
            nc.gpsimd.dma_start(out=s_rows_sb[:, :384], in_=s_rows[:, :384])
            nc.sync.dma_start(out=aug_j_sb[:, 512:1024], in_=aug_j[:, 512:1024])
            nc.sync.dma_start(out=aug_j_sb[:, 1024:1536], in_=aug_j[:, 1024:1536])
            nc.gpsimd.dma_start(out=s_rows_sb[:, 1152:], in_=s_rows[:, 1152:])
            nc.sync.dma_start(out=aug_j_sb[:, 1536:2048], in_=aug_j[:, 1536:2048])
            nc.gpsimd.dma_start(out=s_rows8_sb[:, 8:, :], in_=s_rows8[:, 8 * 48 :])
            nc.sync.dma_start(out=s_rows_sb[:, 384:768], in_=s_rows[:, 384:768])
            nc.sync.dma_start(out=aug_j_sb[:, 2048:2560], in_=aug_j[:, 2048:2560])
            nc.gpsimd.dma_start(out=aug_j_sb[:, 2560:3072], in_=aug_j[:, 2560:3072])
            nc.sync.dma_start(out=s_rows_sb[:, 768:1152], in_=s_rows[:, 768:1152])
            nc.gpsimd.dma_start(out=aug_j_sb[:, 3584:], in_=aug_j[:, 3584:])
            nc.sync.dma_start(out=aug_j_sb[:, 3072:3584], in_=aug_j[:, 3072:3584])
            nc.gpsimd.dma_start(out=aug_i_sb[:, 1024:2048], in_=aug_i[:, 1024:2048])
            nc.sync.dma_start(out=aug_i_sb[:, CHUNK:1024], in_=aug_i[:, CHUNK:1024])
            nc.sync.dma_start(out=s_nat_sb[:], in_=s_nat[:])
            nc.gpsimd.dma_start(out=aug_i_sb[:, 2048:], in_=aug_i[:, 2048:])

            ident = cpool.tile([128, 128], FP16)
            make_identity(nc, ident[:])
            ident8 = cpool.tile([128, 128], FP8)
            make_identity(nc, ident8[:])

            # persistent transposed-mirror buffers, one per owned pair
            t2_tiles = {}
            for (a, b) in VPAIRS:
                dt = FP8 if (a, b) in FP8_PAIRS else FP16
                t2_tiles[(a, b)] = t2pool.tile(
                    [128, 4, CHUNK], dt, name=f"t2_{a}{b}"
                )

            zt = cpool.tile([128, 640], FP16)
            nc.vector.memset(zt[:], 0.0)
            # dedicated spare PSUM slot for warm-up/filler matmuls (shares the
            # acc tag; real accumulators rotate through the other slot first)
            wz = apool.tile([128, CHUNK], F32, tag="acc", name="wz")

            def filler(n):
                for _ in range(n):
                    nc.tensor.matmul(
                        wz[:],
                        lhsT=zt[:, :128],
                        rhs=zt[:, 128:640],
                        start=True,
                        stop=True,
                    )

            # PE warm-up during input DMAs so HAM reaches full clock before
            # the first real MM1; sized to end right as the bulk input lands
            filler(WARMUP_FILLERS)

            # ---------------- production machinery ----------------
            wt_tiles = [None] * NDUO        # duo idx -> SBUF fp16 tile
            duo_emitted = 0
            group_tiles_expd = [0] * len(PROD_GROUPS)

            def wt_slice(ti, lo, hi):
                """W-values slice [128, lo:hi] of production tile ti."""
                d, slot = divmod(ti, GS)
                return wt_tiles[d][:, slot, lo:hi]

            def emit_transposes(gi):
                kind, a, b = PROD_GROUPS[gi]
                if kind != "pair" or (a, b) in RECOMP:
                    return
                t2 = t2_tiles[(a, b)]
                fp8 = (a, b) in FP8_PAIRS
                # T2 tile p covers [j in block 4a+p, i in chunk b]; its 128-col
                # sub-block q is the transpose of production tile q's columns
                # p*128:(p+1)*128.  Two p-slices batch into one PSUM bank.
                # fp8 transposes must write PSUM with element step 2.
                for half in range(2):
                    if fp8:
                        tp = tpool.tile([128, 2, 2 * CHUNK], FP8, tag="tp", name="tp")
                    else:
                        tp = tpool.tile([128, 2, CHUNK], FP16, tag="tp", name="tp")
                    for pp in range(2):
                        p = 2 * half + pp
                        for q in range(4):
                            if fp8:
                                dst = tp[:, pp, 2 * q * JB : 2 * (q + 1) * JB : 2]
                            else:
                                dst = tp[:, pp, q * JB : (q + 1) * JB]
                            nc.tensor.transpose(
                                dst,
                                wt_slice(4 * gi + q, p * JB, (p + 1) * JB),
                                ident8[:] if fp8 else ident[:],
                            )
                    src = tp[:, :, ::2] if fp8 else tp[:]
                    nc.vector.tensor_copy(
                        out=t2[:, 2 * half : 2 * half + 2, :], in_=src
                    )

            def emit_duo():
                nonlocal duo_emitted
                t = duo_emitted
                duo_emitted += 1
                gi_t = PROD_TILES[GS * t][2]
                p = ppool.tile([128, GS, CHUNK], F32, tag="p", name="p")
                for s in range(GS):
                    vjb, ic, _gi = PROD_TILES[GS * t + s]
                    nc.tensor.matmul(
                        p[:, s, :],
                        lhsT=aug_j_sb[32 * s : 32 * s + KAUG, vjb * JB : (vjb + 1) * JB],
                        rhs=aug_i_sb[32 * s : 32 * s + KAUG, ic * CHUNK : (ic + 1) * CHUNK],
                        start=True,
                        stop=True,
                    )
                wt = wpool.tile([128, GS, CHUNK], group_dt(gi_t), tag="w", name="wt")
                nc.scalar.activation(
                    wt[:], p[:], mybir.ActivationFunctionType.Exp
                )
                wt_tiles[t] = wt
                # emit PE transposes for any pair group fully exp'd now
                for s in range(GS):
                    _vjb, _ic, gi = PROD_TILES[GS * t + s]
                    group_tiles_expd[gi] += 1
                    if group_tiles_expd[gi] == 4:
                        emit_transposes(gi)

            def ensure_produced(tile_idx, lookahead=1):
                want = min(tile_idx // GS + 1 + lookahead, NDUO)
                while duo_emitted < want:
                    emit_duo()

            # ---------------- consumption schedule ----------------
            mirrors_at = {c: [] for c in range(NCHUNK)}
            for (a, b) in VPAIRS:
                if (a, b) not in RECOMP:
                    mirrors_at[b].append((a, b))

            mm2_since_duo = [0]

            def maybe_pull_ahead():
                # keep ACT fed during windows with little/no production
                if duo_emitted < NDUO and mm2_since_duo[0] >= 4:
                    emit_duo()
                    mm2_since_duo[0] = 0

            for c in range(NCHUNK):
                # entries: ("direct", vjb, ti) fp16 per-tile;
                #          ("mirror", vjb, (a,b,p)) fp16 per-tile;
                #          ("dr", duo_t, pr) fp8 DoubleRow direct per-duo;
                #          ("drm", (a,b,u), pr) fp8 DoubleRow mirror per-duo.
                mm2s = []
                for gi, (kind, a, b) in enumerate(PROD_GROUPS):
                    if a != c:
                        continue
                    if group_dt(gi) == FP8:
                        for u in range(2):
                            duo_t = 2 * gi + u
                            vjb0 = PROD_TILES[GS * duo_t][0]
                            mm2s.append(("dr", duo_t, vjb0))
                    else:
                        for q in range(4):
                            ti = 4 * gi + q
                            vjb, _ic, _g = PROD_TILES[ti]
                            mm2s.append(("direct", vjb, ti))
                for (a, b) in mirrors_at[c]:
                    if (a, b) in FP8_PAIRS:
                        for u in range(2):
                            mm2s.append(("drm", (a, b, u), 4 * a + 2 * u))
                    else:
                        for p in range(4):
                            mm2s.append(("mirror", 4 * a + p, (a, b, p)))

                acc = apool.tile([48, CHUNK], F32, tag="acc", name="acc")
                nmm = len(mm2s)
                for k, (kind, ref, aux) in enumerate(mm2s):
                    if c == 0 and k == 0:
                        filler(BRIDGE_FILLERS)  # bridge the pipeline-fill PE gap
                    if kind == "direct":
                        vjb, ti = ref, aux
                        ensure_produced(ti)
                        nc.tensor.matmul(
                            acc[:],
                            lhsT=s_rows_sb[:, vjb * 48 : (vjb + 1) * 48],
                            rhs=wt_slice(ti, 0, CHUNK),
                            start=(k == 0),
                            stop=(k == nmm - 1),
                            skip_group_check=True,
                        )
                    elif kind == "mirror":
                        vjb = ref
                        a, b, p = aux
                        nc.tensor.matmul(
                            acc[:],
                            lhsT=s_rows_sb[:, vjb * 48 : (vjb + 1) * 48],
                            rhs=t2_tiles[(a, b)][:, p, :],
                            start=(k == 0),
                            stop=(k == nmm - 1),
                            skip_group_check=True,
                        )
                    elif kind == "dr":
                        duo_t, vjb0 = ref, aux
                        ensure_produced(GS * duo_t + 1)
                        nc.tensor.matmul(
                            acc[:],
                            lhsT=s_rows8_sb[:, vjb0 : vjb0 + 2, :],
                            rhs=wt_tiles[duo_t][:],
                            start=(k == 0),
                            stop=(k == nmm - 1),
                            perf_mode=mybir.MatmulPerfMode.DoubleRow,
                            skip_group_check=True,
                        )
                    else:  # drm
                        a, b, u = ref
                        vjb0 = aux
                        nc.tensor.matmul(
                            acc[:],
                            lhsT=s_rows8_sb[:, vjb0 : vjb0 + 2, :],
                            rhs=t2_tiles[(a, b)][:, 2 * u : 2 * u + 2, :],
                            start=(k == 0),
                            stop=(k == nmm - 1),
                            perf_mode=mybir.MatmulPerfMode.DoubleRow,
                            skip_group_check=True,
                        )
                    mm2_since_duo[0] += 2 if kind in ("dr", "drm") else 1
                    maybe_pull_ahead()

                isl = slice(c * CHUNK, (c + 1) * CHUNK)
                has_fp8 = any(kind in ("dr", "drm") for kind, _r, _a in mm2s)
                if has_fp8:
                    # acc rows 32:48 carry the fp8 s-lo partials
                    t0 = opool.tile([16, CHUNK], F32, tag="t0", name="t0")
                    nc.vector.tensor_sub(t0[:], acc[0:16, :], s_nat_sb[:, isl])
                    o = opool.tile([16, CHUNK], F32, tag="o", name="o")
                    nc.vector.tensor_add(o[:], t0[:], acc[32:48, :])
                else:
                    # fp16 s-lo columns are zeroed: single-op drain
                    o = opool.tile([16, CHUNK], F32, tag="o", name="o")
                    nc.vector.tensor_sub(o[:], acc[0:16, :], s_nat_sb[:, isl])
                nc.sync.dma_start(out=out[:, isl], in_=o[:])

    nc.finalize()
    return nc


def _hi_lo(x: np.ndarray):
    hi = x.astype(np.float16)
    lo = (x - hi.astype(np.float32)).astype(np.float16)
    return hi, lo


def _tri8(x: np.ndarray):
    """Triple-split into e4m3 levels: x ~= a + b + c with rel err ~2^-12."""
    import ml_dtypes

    e4 = ml_dtypes.float8_e4m3
    a = x.astype(e4)
    b = (x - a.astype(np.float32)).astype(e4)
    c = (x - a.astype(np.float32) - b.astype(np.float32)).astype(e4)
    return a, b, c


def _pack_dr_features(lhs36: np.ndarray, rhs36: np.ndarray):
    """Pack [36, N] fp8 feature matrices into the DoubleRow SBUF layouts:
    aug_j8[32s+p, vjb*256 + h*128 + c] = lhs36[18h+p, vjb*128+c]  (strips s=0,1)
    aug_i8[32s+p, ic*1024 + h*512 + c] = rhs36[18h+p, ic*512+c]."""
    import ml_dtypes

    e4 = ml_dtypes.float8_e4m3
    n = lhs36.shape[1]
    aj = np.zeros((128, 2 * n), e4)
    ai = np.zeros((128, 2 * n), e4)
    A = lhs36.reshape(2, 18, n // 128, 128)        # [h, p, blk, c]
    Aj = np.transpose(A, (1, 2, 0, 3)).reshape(18, 2 * n)
    B = rhs36.reshape(2, 18, n // 512, 512)        # [h, p, chunk, c]
    Bi = np.transpose(B, (1, 2, 0, 3)).reshape(18, 2 * n)
    for s in range(2):
        aj[32 * s : 32 * s + 18] = Aj
        ai[32 * s : 32 * s + 18] = Bi
    return aj, ai


def prep_core_inputs(src: np.ndarray, guide: np.ndarray) -> list[dict]:
    """Shard full inputs into the 8 per-core input maps (host-side layout
    prep).  Core parity h sees super-columns permuted by v -> (v+h) % 8."""
    in_maps = []
    for bi in range(BS):
        refs = np.ascontiguousarray(guide[bi].reshape(D, N), dtype=np.float32)
        srcs = np.ascontiguousarray(src[bi].reshape(L, N), dtype=np.float32)
        sq = (refs.astype(np.float64) ** 2).sum(0)
        r_hi, r_lo = _hi_lo(refs)
        q_hi, q_lo = _hi_lo((-0.5 * sq).astype(np.float32))
        ones = np.ones((1, N), np.float16)
        augj = np.concatenate(
            [r_hi, r_lo, r_hi, q_hi[None], q_lo[None], ones, ones], axis=0
        )
        augi = np.concatenate(
            [r_hi, r_hi, r_lo, ones, ones, q_hi[None], q_lo[None]], axis=0
        )
        import ml_dtypes

        e4 = ml_dtypes.float8_e4m3fn
        s_hi, s_lo = _hi_lo(srcs)
        s8h = srcs.astype(e4)
        s8l = (srcs - s8h.astype(np.float32)).astype(e4)
        s_rows_phys = np.zeros((128, NJB * 48), np.float16)
        s_rows8_phys = np.zeros((128, NJB * 48), e4)
        for jb in range(NJB):
            blk = slice(jb * JB, (jb + 1) * JB)
            s_rows_phys[:, 48 * jb : 48 * jb + 16] = s_hi[:, blk].T
            s_rows8_phys[:, 48 * jb : 48 * jb + 16] = s8h[:, blk].T
            s_rows8_phys[:, 48 * jb + 32 : 48 * jb + 48] = s8l[:, blk].T
        for h in range(2):
            perm = [(v + h) % NCHUNK for v in range(NCHUNK)]  # virtual->physical
            pix = np.concatenate(
                [np.arange(p * CHUNK, (p + 1) * CHUNK) for p in perm]
            )
            augj_v = np.zeros((64, N), np.float16)
            augi_v = np.zeros((64, N), np.float16)
            for st in range(2):
                augj_v[32 * st : 32 * st + KAUG] = augj[:, pix]
                augi_v[32 * st : 32 * st + KAUG] = augi[:, pix]
            s_rows_v = np.zeros((128, NJB * 48), np.float16)
            s_rows8_v = np.zeros((128, NJB * 48), e4)
            for v in range(NCHUNK):
                pcol = perm[v]
                s_rows_v[:, v * 4 * 48 : (v + 1) * 4 * 48] = s_rows_phys[
                    :, pcol * 4 * 48 : (pcol + 1) * 4 * 48
                ]
                s_rows8_v[:, v * 4 * 48 : (v + 1) * 4 * 48] = s_rows8_phys[
                    :, pcol * 4 * 48 : (pcol + 1) * 4 * 48
                ]
            # the identity term -src is subtracted by the parity-0 core only
            # (host ADDS the two cores' partials)
            s_nat_v = (
                np.ascontiguousarray(srcs[:, pix]).astype(np.float16)
                if h == 0
                else np.zeros((L, N), np.float16)
            )
            in_maps.append(
                {
                    "aug_j": augj_v,
                    "aug_i": augi_v,
                    "s_rows": s_rows_v,
                    "s_rows8": s_rows8_v,
                    "s_nat": s_nat_v,
                }
            )
    return in_maps


_NC_CACHE = None


def _get_nc() -> bass.Bass:
    global _NC_CACHE
    if _NC_CACHE is None:
        _NC_CACHE = build_nc()
    return _NC_CACHE


def run_on_hw(in_maps, **kwargs):
    return run_bass_kernel_spmd(_get_nc(), in_maps, core_ids=list(range(NCORES)), **kwargs)


def assemble_output(results: list[dict]) -> np.ndarray:
    out = np.zeros((BS, L, N), np.float32)
    for bi in range(BS):
        for h in range(2):
            perm = [(v + h) % NCHUNK for v in range(NCHUNK)]
            part = results[2 * bi + h]["out"]
            for v in range(NCHUNK):
                p = perm[v]
                out[bi, :, p * CHUNK : (p + 1) * CHUNK] += part[
                    :, v * CHUNK : (v + 1) * CHUNK
                ]
    return out.reshape(BS, L, H, W)


def kernel(src_imgs: np.ndarray, guide_imgs: np.ndarray) -> np.ndarray:
    src = np.asarray(src_imgs, dtype=np.float32)
    guide = np.asarray(guide_imgs, dtype=np.float32)
    in_maps = prep_core_inputs(src, guide)
    res = run_on_hw(in_maps)
    return assemble_output(res.results)



# revision 41
# speedup vs baseline: 1.0605x; 1.0251x over previous
"""Trainium2 Bass kernel for BatchedAdjacency (exact Gaussian-kernel MVM).

Math per batch b (n = H*W = 4096 pixels, d = 5 guide dims, L = 16 channels):
    W[i,j]   = exp(-0.5 * ||r_i - r_j||^2)      (symmetric!)
    out[l,i] = sum_j W[i,j] * s[j,l] - s[i,l]

Distribution: 8 cores = 4 batches x 2 cores; each core computes HALF of the
symmetric W (one member of every transpose-pair of 512x512 super-blocks) and
produces a PARTIAL out[16, 4096]; the host adds the two cores' partials.

Symmetry exploitation (the big win vs computing all of W):
  - The 8x8 grid of [512,512] super-blocks has 28 off-diagonal transpose
    pairs + 8 diagonal supers.  A core owns 14 pairs + 4 diagonals.  For an
    owned pair {a<b} it computes T1 = W[j in b, i in a] once via MM1+exp,
    feeds acc[a] directly, and obtains the mirrored position (a,b)
    via PE-transpose (128x128 fp16 identity matmuls) + DVE
    PSUM->SBUF copies, feeding acc[b] later.  exp volume drops ~2x (ACT was
    co-critical with PE in the non-symmetric version).
  - SPMD trick: both cores of a batch run the SAME program; core parity h
    relabels super-columns by v -> (v+h) % 8 in its host-prepped inputs.
    The owned virtual pair set {u, u+d mod 8} (d=1,2,3, u even) + {0,4},{2,6}
    and virtual diagonals {0,2,4,6} tile the physical pair/diag sets exactly
    across the two parities.  The parity-0 core alone subtracts src (the
    identity term); the host ADDS the two partials.

Device pipeline per core (matmuls fp16 with hi/lo splits, PSUM fp32):
  - MM1 computes -0.5*d2 tiles [j=128, i=512] via augmented features (K=19);
    two j-tiles packed into 32-row PE strips fill a 2-bank PSUM duo.
  - ScalarE exp() on duos PSUM->SBUF fp16.
  - PE transposes mirror each pair's 4 tiles (16 blocks) into 2 PSUM banks;
    DVE copies them to the persistent T2 SBUF buffer.
  - MM2 accumulates acc[c][48, 512] over the 8 chunk windows sequentially
    (acc double-buffered in 2 PSUM banks; duos 4; transposes 2).
  - VectorE drain: out = acc_hi + acc_lo - src, DMA to HBM.
"""

import sys

if "/opt/trn_rl_repo" not in sys.path:
    sys.path.insert(0, "/opt/trn_rl_repo")

import numpy as np

import concourse.bacc as bacc
import concourse.bass as bass
import concourse.mybir as mybir
import concourse.tile as tile
from concourse.bass_utils import run_bass_kernel_spmd
from concourse.masks import make_identity


def install_ntff_hook() -> None:
    """Provide antenv.axon_hooks (absent in this image) so that
    run_bass_kernel_spmd can profile via the axon .so when tracing is
    requested -- and so a stray BASS_TRACE env var cannot crash the run."""
    import types

    if "antenv.axon_hooks" in sys.modules:
        return
    hook = None
    try:
        import antenv
        from trn_agent_boot.trn_boot import _ntff_profile_via_ctypes

        hook = _ntff_profile_via_ctypes("/opt/axon/libaxon_pjrt.so")
    except Exception:
        antenv = None
    mod = types.ModuleType("antenv.axon_hooks")
    mod._hook = hook
    mod.get_axon_ntff_profile_hook = lambda: mod._hook
    mod.set_axon_ntff_profile_hook = lambda h: setattr(mod, "_hook", h)
    sys.modules["antenv.axon_hooks"] = mod
    if antenv is not None:
        antenv.axon_hooks = mod


install_ntff_hook()

BS, L, D, H, W = 4, 16, 5, 64, 64
N = H * W            # 4096 pixels
NCORES = 8
CHUNK = 512          # i-tile (PSUM bank / fp32 matmul free-dim limit)
NCHUNK = N // CHUNK  # 8 super-columns, all owned by every core (partial sums)
JB = 128             # j-block (contraction tile)
NJB = N // JB        # 32
KAUG = 19            # augmented feature count (fp16 path, unused)
K8 = 18              # fp8 DoubleRow lanes per half (36 features total)
GS = 2               # j-tiles packed per PSUM duo tile

FP16 = mybir.dt.float16
FP8 = mybir.dt.float8e4
F32 = mybir.dt.float32

# ---- virtual super-block ownership (same for every core; the host permutes
# ---- columns per core parity so the union covers the full symmetric grid).
VPAIRS = []
for d in (1, 2, 3):
    for u in (0, 2, 4, 6):
        a, b = u, (u + d) % 8
        VPAIRS.append((min(a, b), max(a, b)))
VPAIRS += [(0, 4), (2, 6)]
VDIAGS = [0, 2, 4, 6]
# pairs whose mirror is RECOMPUTED by a second MM1+exp at window b instead of
# PE-transposed: per pair this trades 1.66us of PE transposes for 0.85us of
# MM1 plus ~2.1us of exp on the slack ScalarE (ACT stays ~20us under PE)
RECOMP = {(0, 1), (2, 3)}
# pairs whose W tiles are exp'd straight to fp8e4m3 and consumed by
# DoubleRow MM2 (2 j-blocks per instruction, ~2x MM2 rate on those tiles).
# 4 pairs = 16/64 supers across the two parities; exact-input simulation
# puts the end-to-end rel err at 1.33e-2 vs the 2e-2 gate.
FP8_PAIRS = {(2, 4), (2, 6), (4, 6), (2, 5)}
assert FP8_PAIRS <= set(VPAIRS) and not (FP8_PAIRS & RECOMP)

# production groups per window (pairs {a,b} are produced at window a as
# T1 = W[j in b's blocks, i in a's chunk]); diag c produced at window c.
PROD_GROUPS = []
for c in range(NCHUNK):
    if c in VDIAGS:
        PROD_GROUPS.append(("diag", c, c))
    for (a, b) in VPAIRS:
        if a == c:
            PROD_GROUPS.append(("pair", a, b))
    for (a, b) in sorted(RECOMP):
        if b == c:
            # recomputed mirror position (a, b): tiles (4a+t, i-chunk b)
            PROD_GROUPS.append(("rmirr", b, a))

# flat production tile stream: tile = (vjb, ichunk, group_index); groups are
# 4 tiles so duo (GS=2) boundaries never straddle a group asymmetrically
PROD_TILES = []
for gi, (kind, a, b) in enumerate(PROD_GROUPS):
    for t in range(4):
        PROD_TILES.append((4 * b + t, a, gi))
NT = len(PROD_TILES)            # 72
NDUO = NT // GS                 # 36
assert NT == GS * NDUO


WARMUP_FILLERS = 5
BRIDGE_FILLERS = 3


def build_nc() -> bass.Bass:
    nc = bacc.Bacc()

    # aug tensors carry the 19 features replicated in rows 0:19 and 32:51 of
    # a 64-partition layout (wide DMAs: narrow-partition transfers only get
    # partition_count/128 of DMA bandwidth).
    aug_j = nc.declare_dram_parameter("aug_j", [64, N], FP16, isOutput=False)
    aug_i = nc.declare_dram_parameter("aug_i", [64, N], FP16, isOutput=False)
    s_rows = nc.declare_dram_parameter("s_rows", [128, NJB * 48], FP16, isOutput=False)
    s_rows8 = nc.declare_dram_parameter(
        "s_rows8", [128, NJB * 48], FP8, isOutput=False
    )
    s_nat = nc.declare_dram_parameter("s_nat", [16, N], FP16, isOutput=False)
    out = nc.declare_dram_parameter("out", [16, N], F32, isOutput=True)

    def group_dt(gi: int):
        kind, a, b = PROD_GROUPS[gi]
        return FP8 if (kind == "pair" and (a, b) in FP8_PAIRS) else FP16

    with tile.TileContext(nc) as tc:
        with (
            tc.tile_pool(name="const", bufs=1) as cpool,
            tc.tile_pool(name="t2pool", bufs=1) as t2pool,
            tc.tile_pool(name="wpool", bufs=10) as wpool,
            tc.tile_pool(name="ppool", bufs=2, space="PSUM") as ppool,
            tc.tile_pool(name="apool", bufs=2, space="PSUM") as apool,
            tc.tile_pool(name="tpool", bufs=2, space="PSUM") as tpool,
            tc.tile_pool(name="opool", bufs=2) as opool,
        ):
            aug_j_sb = cpool.tile([64, N], FP16)
            aug_i_sb = cpool.tile([64, N], FP16)
            s_rows_sb = cpool.tile([128, NJB * 48], FP16)
            s_rows8_sb = cpool.tile([128, NJB, 48], FP8)
            s_nat_sb = cpool.tile([16, N], FP16)
            # input DMAs spread over the 3 DMA-capable queues (sync, scalar,
            # gpsimd), ordered so window 0's operands land first: aug_i
            # chunk 0, aug_j in consumption order, s_rows; the rest trails.
            # first pieces sized >=1KB per partition line (smaller lines lose
            # DMA efficiency); dependency granularity = one dma_start
            # dma_start triggers occupy the ISSUING engine's queue (~0.6us
            # each): scalar (= ACT, the production pacer) gets only 2 early
            # triggers; sync and gpsimd carry the rest, ordered by need-time.
            nc.scalar.dma_start(out=aug_j_sb[:, :512], in_=aug_j[:, :512])
            nc.scalar.dma_start(out=s_rows8_sb[:, :8, :], in_=s_rows8[:, : 8 * 48])
            nc.sync.dma_start(out=aug_i_sb[:, :CHUNK], in_=aug_i[:, :CHUNK])
            nc.gpsimd.dma_start(out=s_rows_sb[:, :384], in_=s_rows[:, :384])
            nc.sync.dma_start(out=aug_j_sb[:, 512:1024], in_=aug_j[:, 512:1024])
            nc.sync.dma_start(out=aug_j_sb[:, 1024:1536], in_=aug_j[:, 1024:1536])
            nc.gpsimd.dma_start(out=s_rows_sb[:, 1152:], in_=s_rows[:, 1152:])
            nc.sync.dma_start(out=aug_j_sb[:, 1536:2048], in_=aug_j[:, 1536:2048])
            nc.gpsimd.dma_start(out=s_rows8_sb[:, 8:, :], in_=s_rows8[:, 8 * 48 :])
            nc.sync.dma_start(out=s_rows_sb[:, 384:768], in_=s_rows[:, 384:768])
            nc.sync.dma_start(out=aug_j_sb[:, 2048:2560], in_=aug_j[:, 2048:2560])
            nc.gpsimd.dma_start(out=aug_j_sb[:, 2560:3072], in_=aug_j[:, 2560:3072])
            nc.sync.dma_start(out=s_rows_sb[:, 768:1152], in_=s_rows[:, 768:1152])
            nc.gpsimd.dma_start(out=aug_j_sb[:, 3584:], in_=aug_j[:, 3584:])
            nc.sync.dma_start(out=aug_j_sb[:, 3072:3584], in_=aug_j[:, 3072:3584])
            nc.gpsimd.dma_start(out=aug_i_sb[:, 1024:2048], in_=aug_i[:, 1024:2048])
            nc.sync.dma_start(out=aug_i_sb[:, CHUNK:1024], in_=aug_i[:, CHUNK:1024])
            nc.sync.dma_start(out=s_nat_sb[:], in_=s_nat[:])
            nc.gpsimd.dma_start(out=aug_i_sb[:, 2048:], in_=aug_i[:, 2048:])

            ident = cpool.tile([128, 128], FP16)
            make_identity(nc, ident[:])
            ident8 = cpool.tile([128, 128], FP8)
            make_identity(nc, ident8[:])

            # persistent transposed-mirror buffers, one per owned pair
            t2_tiles = {}
            for (a, b) in VPAIRS:
                dt = FP8 if (a, b) in FP8_PAIRS else FP16
                t2_tiles[(a, b)] = t2pool.tile(
                    [128, 4, CHUNK], dt, name=f"t2_{a}{b}"
                )

            zt = cpool.tile([128, 640], FP16)
            nc.vector.memset(zt[:], 0.0)
            # dedicated spare PSUM slot for warm-up/filler matmuls (shares the
            # acc tag; real accumulators rotate through the other slot first)
            wz = apool.tile([128, CHUNK], F32, tag="acc", name="wz")

            def filler(n):
                for _ in range(n):
                    nc.tensor.matmul(
                        wz[:],
                        lhsT=zt[:, :128],
                        rhs=zt[:, 128:640],
                        start=True,
                        stop=True,
                    )

            # PE warm-up during input DMAs so HAM reaches full clock before
            # the first real MM1; sized to end right as the bulk input lands
            filler(WARMUP_FILLERS)

            # ---------------- production machinery ----------------
            wt_tiles = [None] * NDUO        # duo idx -> SBUF fp16 tile
            duo_emitted = 0
            group_tiles_expd = [0] * len(PROD_GROUPS)

            def wt_slice(ti, lo, hi):
                """W-values slice [128, lo:hi] of production tile ti."""
                d, slot = divmod(ti, GS)
                return wt_tiles[d][:, slot, lo:hi]

            def emit_transposes(gi):
                kind, a, b = PROD_GROUPS[gi]
                if kind != "pair" or (a, b) in RECOMP:
                    return
                t2 = t2_tiles[(a, b)]
                fp8 = (a, b) in FP8_PAIRS
                # T2 tile p covers [j in block 4a+p, i in chunk b]; its 128-col
                # sub-block q is the transpose of production tile q's columns
                # p*128:(p+1)*128.  Two p-slices batch into one PSUM bank.
                # fp8 transposes must write PSUM with element step 2.
                for half in range(2):
                    if fp8:
                        tp = tpool.tile([128, 2, 2 * CHUNK], FP8, tag="tp", name="tp")
                    else:
                        tp = tpool.tile([128, 2, CHUNK], FP16, tag="tp", name="tp")
                    for pp in range(2):
                        p = 2 * half + pp
                        for q in range(4):
                            if fp8:
                                dst = tp[:, pp, 2 * q * JB : 2 * (q + 1) * JB : 2]
                            else:
                                dst = tp[:, pp, q * JB : (q + 1) * JB]
                            nc.tensor.transpose(
                                dst,
                                wt_slice(4 * gi + q, p * JB, (p + 1) * JB),
                                ident8[:] if fp8 else ident[:],
                            )
                    src = tp[:, :, ::2] if fp8 else tp[:]
                    nc.vector.tensor_copy(
                        out=t2[:, 2 * half : 2 * half + 2, :], in_=src
                    )

            def emit_duo():
                nonlocal duo_emitted
                t = duo_emitted
                duo_emitted += 1
                gi_t = PROD_TILES[GS * t][2]
                p = ppool.tile([128, GS, CHUNK], F32, tag="p", name="p")
                for s in range(GS):
                    vjb, ic, _gi = PROD_TILES[GS * t + s]
                    nc.tensor.matmul(
                        p[:, s, :],
                        lhsT=aug_j_sb[32 * s : 32 * s + KAUG, vjb * JB : (vjb + 1) * JB],
                        rhs=aug_i_sb[32 * s : 32 * s + KAUG, ic * CHUNK : (ic + 1) * CHUNK],
                        start=True,
                        stop=True,
                    )
                wt = wpool.tile([128, GS, CHUNK], group_dt(gi_t), tag="w", name="wt")
                nc.scalar.activation(
                    wt[:], p[:], mybir.ActivationFunctionType.Exp
                )
                wt_tiles[t] = wt
                # emit PE transposes for any pair group fully exp'd now
                for s in range(GS):
                    _vjb, _ic, gi = PROD_TILES[GS * t + s]
                    group_tiles_expd[gi] += 1
                    if group_tiles_expd[gi] == 4:
                        emit_transposes(gi)

            def ensure_produced(tile_idx, lookahead=1):
                want = min(tile_idx // GS + 1 + lookahead, NDUO)
                while duo_emitted < want:
                    emit_duo()

            # ---------------- consumption schedule ----------------
            mirrors_at = {c: [] for c in range(NCHUNK)}
            for (a, b) in VPAIRS:
                if (a, b) not in RECOMP:
                    mirrors_at[b].append((a, b))

            mm2_since_duo = [0]

            def maybe_pull_ahead():
                # keep ACT fed during windows with little/no production
                if duo_emitted < NDUO and mm2_since_duo[0] >= 4:
                    emit_duo()
                    mm2_since_duo[0] = 0

            for c in range(NCHUNK):
                # entries: ("direct", vjb, ti) fp16 per-tile;
                #          ("mirror", vjb, (a,b,p)) fp16 per-tile;
                #          ("dr", duo_t, pr) fp8 DoubleRow direct per-duo;
                #          ("drm", (a,b,u), pr) fp8 DoubleRow mirror per-duo.
                mm2s = []
                for gi, (kind, a, b) in enumerate(PROD_GROUPS):
                    if a != c:
                        continue
                    if group_dt(gi) == FP8:
                        for u in range(2):
                            duo_t = 2 * gi + u
                            vjb0 = PROD_TILES[GS * duo_t][0]
                            mm2s.append(("dr", duo_t, vjb0))
                    else:
                        for q in range(4):
                            ti = 4 * gi + q
                            vjb, _ic, _g = PROD_TILES[ti]
                            mm2s.append(("direct", vjb, ti))
                for (a, b) in mirrors_at[c]:
                    if (a, b) in FP8_PAIRS:
                        for u in range(2):
                            mm2s.append(("drm", (a, b, u), 4 * a + 2 * u))
                    else:
                        for p in range(4):
                            mm2s.append(("mirror", 4 * a + p, (a, b, p)))

                acc = apool.tile([48, CHUNK], F32, tag="acc", name="acc")
                nmm = len(mm2s)
                for k, (kind, ref, aux) in enumerate(mm2s):
                    if c == 0 and k == 0:
                        filler(BRIDGE_FILLERS)  # bridge the pipeline-fill PE gap
                    if kind == "direct":
                        vjb, ti = ref, aux
                        ensure_produced(ti)
                        nc.tensor.matmul(
                            acc[:],
                            lhsT=s_rows_sb[:, vjb * 48 : (vjb + 1) * 48],
                            rhs=wt_slice(ti, 0, CHUNK),
                            start=(k == 0),
                            stop=(k == nmm - 1),
                            skip_group_check=True,
                        )
                    elif kind == "mirror":
                        vjb = ref
                        a, b, p = aux
                        nc.tensor.matmul(
                            acc[:],
                            lhsT=s_rows_sb[:, vjb * 48 : (vjb + 1) * 48],
                            rhs=t2_tiles[(a, b)][:, p, :],
                            start=(k == 0),
                            stop=(k == nmm - 1),
                            skip_group_check=True,
                        )
                    elif kind == "dr":
                        duo_t, vjb0 = ref, aux
                        ensure_produced(GS * duo_t + 1)
                        nc.tensor.matmul(
                            acc[:],
                            lhsT=s_rows8_sb[:, vjb0 : vjb0 + 2, :],
                            rhs=wt_tiles[duo_t][:],
                            start=(k == 0),
                            stop=(k == nmm - 1),
                            perf_mode=mybir.MatmulPerfMode.DoubleRow,
                            skip_group_check=True,
                        )
                    else:  # drm
                        a, b, u = ref
                        vjb0 = aux
                        nc.tensor.matmul(
                            acc[:],
                            lhsT=s_rows8_sb[:, vjb0 : vjb0 + 2, :],
                            rhs=t2_tiles[(a, b)][:, 2 * u : 2 * u + 2, :],
                            start=(k == 0),
                            stop=(k == nmm - 1),
                            perf_mode=mybir.MatmulPerfMode.DoubleRow,
                            skip_group_check=True,
                        )
                    mm2_since_duo[0] += 2 if kind in ("dr", "drm") else 1
                    maybe_pull_ahead()

                isl = slice(c * CHUNK, (c + 1) * CHUNK)
                has_fp8 = any(kind in ("dr", "drm") for kind, _r, _a in mm2s)
                if has_fp8:
                    # acc rows 32:48 carry the fp8 s-lo partials
                    t0 = opool.tile([16, CHUNK], F32, tag="t0", name="t0")
                    nc.vector.tensor_sub(t0[:], acc[0:16, :], s_nat_sb[:, isl])
                    o = opool.tile([16, CHUNK], F32, tag="o", name="o")
                    nc.vector.tensor_add(o[:], t0[:], acc[32:48, :])
                else:
                    # fp16 s-lo columns are zeroed: single-op drain
                    o = opool.tile([16, CHUNK], F32, tag="o", name="o")
                    nc.vector.tensor_sub(o[:], acc[0:16, :], s_nat_sb[:, isl])
                nc.sync.dma_start(out=out[:, isl], in_=o[:])

    nc.finalize()
    return nc


def _hi_lo(x: np.ndarray):
    hi = x.astype(np.float16)
    lo = (x - hi.astype(np.float32)).astype(np.float16)
    return hi, lo


def _tri8(x: np.ndarray):
    """Triple-split into e4m3 levels: x ~= a + b + c with rel err ~2^-12."""
    import ml_dtypes

    e4 = ml_dtypes.float8_e4m3
    a = x.astype(e4)
    b = (x - a.astype(np.float32)).astype(e4)
    c = (x - a.astype(np.float32) - b.astype(np.float32)).astype(e4)
    return a, b, c


def _pack_dr_features(lhs36: np.ndarray, rhs36: np.ndarray):
    """Pack [36, N] fp8 feature matrices into the DoubleRow SBUF layouts:
    aug_j8[32s+p, vjb*256 + h*128 + c] = lhs36[18h+p, vjb*128+c]  (strips s=0,1)
    aug_i8[32s+p, ic*1024 + h*512 + c] = rhs36[18h+p, ic*512+c]."""
    import ml_dtypes

    e4 = ml_dtypes.float8_e4m3
    n = lhs36.shape[1]
    aj = np.zeros((128, 2 * n), e4)
    ai = np.zeros((128, 2 * n), e4)
    A = lhs36.reshape(2, 18, n // 128, 128)        # [h, p, blk, c]
    Aj = np.transpose(A, (1, 2, 0, 3)).reshape(18, 2 * n)
    B = rhs36.reshape(2, 18, n // 512, 512)        # [h, p, chunk, c]
    Bi = np.transpose(B, (1, 2, 0, 3)).reshape(18, 2 * n)
    for s in range(2):
        aj[32 * s : 32 * s + 18] = Aj
        ai[32 * s : 32 * s + 18] = Bi
    return aj, ai


def prep_core_inputs(src: np.ndarray, guide: np.ndarray) -> list[dict]:
    """Shard full inputs into the 8 per-core input maps (host-side layout
    prep).  Core parity h sees super-columns permuted by v -> (v+h) % 8."""
    in_maps = []
    for bi in range(BS):
        refs = np.ascontiguousarray(guide[bi].reshape(D, N), dtype=np.float32)
        srcs = np.ascontiguousarray(src[bi].reshape(L, N), dtype=np.float32)
        sq = (refs.astype(np.float64) ** 2).sum(0)
        r_hi, r_lo = _hi_lo(refs)
        q_hi, q_lo = _hi_lo((-0.5 * sq).astype(np.float32))
        ones = np.ones((1, N), np.float16)
        augj = np.concatenate(
            [r_hi, r_lo, r_hi, q_hi[None], q_lo[None], ones, ones], axis=0
        )
        augi = np.concatenate(
            [r_hi, r_hi, r_lo, ones, ones, q_hi[None], q_lo[None]], axis=0
        )
        import ml_dtypes

        e4 = ml_dtypes.float8_e4m3fn
        s_hi, s_lo = _hi_lo(srcs)
        s8h = srcs.astype(e4)
        s8l = (srcs - s8h.astype(np.float32)).astype(e4)
        s_rows_phys = np.zeros((128, NJB * 48), np.float16)
        s_rows8_phys = np.zeros((128, NJB * 48), e4)
        for jb in range(NJB):
            blk = slice(jb * JB, (jb + 1) * JB)
            s_rows_phys[:, 48 * jb : 48 * jb + 16] = s_hi[:, blk].T
            s_rows8_phys[:, 48 * jb : 48 * jb + 16] = s8h[:, blk].T
            s_rows8_phys[:, 48 * jb + 32 : 48 * jb + 48] = s8l[:, blk].T
        for h in range(2):
            perm = [(v + h) % NCHUNK for v in range(NCHUNK)]  # virtual->physical
            pix = np.concatenate(
                [np.arange(p * CHUNK, (p + 1) * CHUNK) for p in perm]
            )
            augj_v = np.zeros((64, N), np.float16)
            augi_v = np.zeros((64, N), np.float16)
            for st in range(2):
                augj_v[32 * st : 32 * st + KAUG] = augj[:, pix]
                augi_v[32 * st : 32 * st + KAUG] = augi[:, pix]
            s_rows_v = np.zeros((128, NJB * 48), np.float16)
            s_rows8_v = np.zeros((128, NJB * 48), e4)
            for v in range(NCHUNK):
                pcol = perm[v]
                s_rows_v[:, v * 4 * 48 : (v + 1) * 4 * 48] = s_rows_phys[
                    :, pcol * 4 * 48 : (pcol + 1) * 4 * 48
                ]
                s_rows8_v[:, v * 4 * 48 : (v + 1) * 4 * 48] = s_rows8_phys[
                    :, pcol * 4 * 48 : (pcol + 1) * 4 * 48
                ]
            # the identity term -src is subtracted by the parity-0 core only
            # (host ADDS the two cores' partials)
            s_nat_v = (
                np.ascontiguousarray(srcs[:, pix]).astype(np.float16)
                if h == 0
                else np.zeros((L, N), np.float16)
            )
            in_maps.append(
                {
                    "aug_j": augj_v,
                    "aug_i": augi_v,
                    "s_rows": s_rows_v,
                    "s_rows8": s_rows8_v,
                    "s_nat": s_nat_v,
                }
            )
    return in_maps


_NC_CACHE = None


def _get_nc() -> bass.Bass:
    global _NC_CACHE
    if _NC_CACHE is None:
        _NC_CACHE = build_nc()
    return _NC_CACHE


def run_on_hw(in_maps, **kwargs):
    return run_bass_kernel_spmd(_get_nc(), in_maps, core_ids=list(range(NCORES)), **kwargs)


def assemble_output(results: list[dict]) -> np.ndarray:
    out = np.zeros((BS, L, N), np.float32)
    for bi in range(BS):
        for h in range(2):
            perm = [(v + h) % NCHUNK for v in range(NCHUNK)]
            part = results[2 * bi + h]["out"]
            for v in range(NCHUNK):
                p = perm[v]
                out[bi, :, p * CHUNK : (p + 1) * CHUNK] += part[
                    :, v * CHUNK : (v + 1) * CHUNK
                ]
    return out.reshape(BS, L, H, W)


def kernel(src_imgs: np.ndarray, guide_imgs: np.ndarray) -> np.ndarray:
    src = np.asarray(src_imgs, dtype=np.float32)
    guide = np.asarray(guide_imgs, dtype=np.float32)
    in_maps = prep_core_inputs(src, guide)
    res = run_on_hw(in_maps)
    return assemble_output(res.results)

